# revision 1
# baseline (speedup 1.0000x reference)
"""Distributed Trainium2 (8 NeuronCores) kernel for the 3-node ConvGRU
message-passing network.

Strategy (memory-bound problem: ~4.8 GB of per-use weight traffic dominates):
  - The five big projection matrices (td_w0/td_w1/bu_w0/bu_w1/bu_w2) are
    tensor-sharded across the 8 cores by output feature (= channel groups),
    host-pre-transposed into [k_chunk, 98, O/8] streaming layout, and DMA-
    streamed from HBM each use -> each byte is read once chip-wide per use.
  - Activations (tiny) are replicated: every core runs the full ConvGRU cell
    for all 16 batch elements. After each sharded matmul, the output shards
    are AllGathered (o-major [O, B] f32 bounce) and reloaded into the padded
    [ch, 16, 16, B] conv layout.
  - Convs: 9 shifted-flat-window accumulating matmuls (K=48) in fp32r.
  - Big matmuls: lhsT = transposed activations [98, B] per (ch, s) feature
    chunk (built with PE transposes), rhs = streamed weight tiles, fp32r.
  - Collectives are the latency hazard (~0.2-0.4 ms fixed cost each, engine-
    serialized), so the node updates are scheduled as rounds needing only
    TWO AllGathers per timestep: B(u)={td1(u),bu1(u)} -> cell1(u);
    A(u)={bu2(u),bu0(u+1),td0(u+1)} -> cell2(u), cell0(u+1), with bu2(u)
    and td0(u+1) sharing one maxpool+transpose of h1@u.
    Measured ~266 us/timestep (vs 157 us/step with collectives stubbed out
    and ~221 us/step HBM-stream roofline).

Self-contained: hardcodes all shapes; host-side numpy does the sharding,
permutation and final unshard.
"""
import sys
import numpy as np

for _p in ("/opt/trn_rl_repo", "/opt/pypackages",
           "/root/.axon_site", "/root/.axon_site/_ro/trn_rl_repo",
           "/root/.axon_site/_ro/pypackages"):
    if _p not in sys.path:
        sys.path.append(_p)

import concourse.bass as bass
import concourse.bacc as bacc
import concourse.mybir as mybir
import concourse.tile as tile
from concourse import bass_utils

F32 = mybir.dt.float32
F32R = mybir.dt.float32r
BF16 = mybir.dt.bfloat16
AF = mybir.ActivationFunctionType
GDT = F32   # dtype of the gather path (bu/td bounce + reload buffers)

NCORES = 8
B, T, C, H, W = 16, 8, 3, 14, 14
HID, IND, N = 32, 16, 3
CIN = IND + HID              # 48 conv input channels
YP = XP = 16                 # padded spatial
# conv valid output flat window (phys coords, (y*XP+x)*B): (1,1)..(14,14)
WSTART = (1 * XP + 1) * B
WLEN = ((14 * XP + 14) - (1 * XP + 1) + 1) * B    # 3552
SHIFTS = [(dy, dx) for dy in (-1, 0, 1) for dx in (-1, 0, 1)]

KP = 98                      # partitions per feature chunk (7 y-rows x 14 x)
KH = 2 * HID                 # 64 chunks for hidden-sized contraction (6272)
KX = 2 * C                   # 6 chunks for x contraction (588)
O_TD = (IND + HID) * H * W   # 9408
O_BU = IND * H * W           # 3136
OTD8 = O_TD // NCORES        # 1176 = 6 channels
OBU8 = O_BU // NCORES        # 392  = 2 channels
NJ_TD = (OTD8 + 127) // 128  # 10 o-chunks
NJ_BU = (OBU8 + 127) // 128  # 4
GRP_TD = 2                   # weight K-chunks per DMA (td)
GRP_BU = 4

_CACHED = {}


# ---------------------------------------------------------------- graph ----
def build_graph(t_end=T + N - 1, debug_h=False, no_cc=False):
    nc = bacc.Bacc(None, target_bir_lowering=False, debug=False,
                   num_devices=NCORES)

    dp = nc.declare_dram_parameter
    # streamed weight shards [k, 98, O/8]
    tw0 = dp("tw0", [KH, KP, OTD8], F32R, isOutput=False)
    tw1 = dp("tw1", [KH, KP, OTD8], F32R, isOutput=False)
    bw0 = dp("bw0", [KX, KP, OBU8], F32R, isOutput=False)
    bw1 = dp("bw1", [KH, KP, OBU8], F32R, isOutput=False)
    bw2 = dp("bw2", [KH, KP, OBU8], F32R, isOutput=False)
    # bias shards (o-chunk padded)
    tb0 = dp("tb0", [NJ_TD, 128], F32, isOutput=False)
    tb1 = dp("tb1", [NJ_TD, 128], F32, isOutput=False)
    bb0 = dp("bb0", [NJ_BU, 128], F32, isOutput=False)
    bb1 = dp("bb1", [NJ_BU, 128], F32, isOutput=False)
    bb2 = dp("bb2", [NJ_BU, 128], F32, isOutput=False)
    # pre-transposed input x: [t, k, 98, B]
    xt_in = dp("xt", [T, KX, KP, B], F32R, isOutput=False)
    # conv weights [node, shift, ci(dev order), co] and biases
    wg_in = dp("wg", [N, 9, CIN, 2 * HID], F32R, isOutput=False)
    wc_in = dp("wc", [N, 9, CIN, HID], F32R, isOutput=False)
    bg_in = dp("bg", [N, 2 * HID], F32, isOutput=False)
    bc_in = dp("bc", [N, HID], F32, isOutput=False)
    # fc
    fc1_in = dp("fc1t", [KH, KP, 100], F32R, isOutput=False)
    fc1b_in = dp("fc1b", [100, 1], F32, isOutput=False)
    fc2_in = dp("fc2t", [100, 10], F32, isOutput=False)
    fc2b_in = dp("fc2b", [10, 1], F32, isOutput=False)
    ident_in = dp("ident", [128, 128], F32, isOutput=False)
    out_ext = dp("out", [10, B], F32, isOutput=True)
    dbg_ext = dp("dbg", [N, HID, 14, 14, B], F32, isOutput=True) if debug_h else None

    from contextlib import ExitStack
    with tile.TileContext(nc) as tc, ExitStack() as ctx:
        consts = ctx.enter_context(tc.tile_pool(name="consts", bufs=1))
        wtd_pool = ctx.enter_context(tc.tile_pool(name="wtd", bufs=3))
        wbu_pool = ctx.enter_context(tc.tile_pool(name="wbu", bufs=3))
        mpt_pool = ctx.enter_context(tc.tile_pool(name="mpt", bufs=4))
        pst_pool = ctx.enter_context(tc.tile_pool(name="pst", bufs=2, space="PSUM"))
        acc_pool = ctx.enter_context(tc.tile_pool(name="accp", bufs=1, space="PSUM"))
        conv_pool = ctx.enter_context(tc.tile_pool(name="convp", bufs=2, space="PSUM"))
        sbacc_pool = ctx.enter_context(tc.tile_pool(name="sbacc", bufs=1))
        outt_pool = ctx.enter_context(tc.tile_pool(name="outt", bufs=2))
        dram = ctx.enter_context(tc.tile_pool(name="dram", bufs=1, space="DRAM"))

        # ---------------- constants ----------------
        ident = consts.tile([128, 128], F32)
        nc.sync.dma_start(ident[:], ident_in[:])
        wg_sb = consts.tile([CIN, N, 9, 2 * HID], F32R)
        nc.sync.dma_start(wg_sb[:], wg_in[:].rearrange("n s c o -> c n s o"))
        wc_sb = consts.tile([CIN, N, 9, HID], F32R)
        nc.sync.dma_start(wc_sb[:], wc_in[:].rearrange("n s c o -> c n s o"))
        bg_sb = consts.tile([2 * HID, N], F32)
        nc.sync.dma_start(bg_sb[:], bg_in[:].rearrange("n o -> o n"))
        bc_sb = consts.tile([HID, N], F32)
        nc.sync.dma_start(bc_sb[:], bc_in[:].rearrange("n o -> o n"))
        tb0_sb = consts.tile([128, NJ_TD], F32)
        nc.sync.dma_start(tb0_sb[:], tb0[:].rearrange("j p -> p j"))
        tb1_sb = consts.tile([128, NJ_TD], F32)
        nc.sync.dma_start(tb1_sb[:], tb1[:].rearrange("j p -> p j"))
        bb0_sb = consts.tile([128, NJ_BU], F32)
        nc.sync.dma_start(bb0_sb[:], bb0[:].rearrange("j p -> p j"))
        bb1_sb = consts.tile([128, NJ_BU], F32)
        nc.sync.dma_start(bb1_sb[:], bb1[:].rearrange("j p -> p j"))
        bb2_sb = consts.tile([128, NJ_BU], F32)
        nc.sync.dma_start(bb2_sb[:], bb2[:].rearrange("j p -> p j"))
        fc2_sb = consts.tile([100, 10], F32)
        nc.sync.dma_start(fc2_sb[:], fc2_in[:])
        fc1b_sb = consts.tile([100, 1], F32)
        nc.sync.dma_start(fc1b_sb[:], fc1b_in[:])
        fc2b_sb = consts.tile([10, 1], F32)
        nc.sync.dma_start(fc2b_sb[:], fc2b_in[:])

        # ------------- dedicated activation tensors (shared/aliased) -------
        h = [consts.tile([HID, YP, XP, B], F32, name=f"h{i}", tag=f"h{i}")
             for i in range(N)]
        comb = consts.tile([CIN, YP, XP, B], F32R)    # conv input (fp32r: matmul operand)
        rz = consts.tile([2 * HID, YP, XP, B], F32)   # gates; [0:HID] doubles
        #   as cand / maxpool output / relu buffer
        bu_buf = consts.tile([IND, YP, XP, B], GDT )  # shared bu reload buffer
        td_buf = [consts.tile([CIN, YP, XP, B], GDT , name=f"td{i}", tag=f"td{i}")
                  for i in range(2)]
        for tt in h + td_buf + [rz, bu_buf]:
            nc.vector.memset(tt[:], 0.0)
        nc.vector.memset(comb[:].bitcast(F32), 0.0)
        mp = rz    # [0:HID] slice used as maxpool result
        tmq = comb  # [0:HID] slice used as maxpool x-pass scratch

        # ---------------- helpers ----------------
        def maxpool_transpose(src):
            """maxpool3x3(SAME) of src[0:HID] -> transposed [98, HID, B] x2."""
            nc.vector.tensor_max(tmq[0:HID, 1:15, 2:14, :], src[0:HID, 1:15, 1:13, :], src[0:HID, 1:15, 2:14, :])
            nc.vector.tensor_max(tmq[0:HID, 1:15, 2:14, :], tmq[0:HID, 1:15, 2:14, :], src[0:HID, 1:15, 3:15, :])
            nc.vector.tensor_max(tmq[0:HID, 1:15, 1:2, :], src[0:HID, 1:15, 1:2, :], src[0:HID, 1:15, 2:3, :])
            nc.vector.tensor_max(tmq[0:HID, 1:15, 14:15, :], src[0:HID, 1:15, 13:14, :], src[0:HID, 1:15, 14:15, :])
            nc.vector.tensor_max(mp[0:HID, 2:14, 1:15, :], tmq[0:HID, 1:13, 1:15, :], tmq[0:HID, 2:14, 1:15, :])
            nc.vector.tensor_max(mp[0:HID, 2:14, 1:15, :], mp[0:HID, 2:14, 1:15, :], tmq[0:HID, 3:15, 1:15, :])
            nc.vector.tensor_max(mp[0:HID, 1:2, 1:15, :], tmq[0:HID, 1:2, 1:15, :], tmq[0:HID, 2:3, 1:15, :])
            nc.vector.tensor_max(mp[0:HID, 14:15, 1:15, :], tmq[0:HID, 13:14, 1:15, :], tmq[0:HID, 14:15, 1:15, :])
            return transpose_feat(mp)

        def transpose_feat(src):
            """src[0:HID] [*, YP, XP, B] -> pair of [98, HID, B] tiles.
            PE transpose needs a single-free-dim input, so first repack the
            (y, x)-strided valid slice contiguously per batch."""
            out = []
            for s in range(2):
                mt = mpt_pool.tile([KP, HID, B], F32R, tag="mpt", name=f"mpt{s}")
                y0 = 1 + 7 * s
                stg = mpt_pool.tile([HID, B, KP], F32, tag="stg", name="stg", bufs=1)
                nc.vector.tensor_copy(
                    stg[:].rearrange("c b (y x) -> c y x b", y=7, x=14),
                    src[0:HID, y0:y0 + 7, 1:15, :])
                for b in range(B):
                    pt = pst_pool.tile([128, HID], F32, tag="psT", name="ptt")
                    nc.tensor.transpose(
                        pt[:KP, 0:HID],
                        stg[:, b, :].opt(),
                        ident[0:HID, 0:HID],
                    )
                    nc.vector.tensor_copy(mt[:, 0:HID, b], pt[:KP, 0:HID])
                out.append(mt)
            return out

        def big_matmul(nk, o8, nj, lhsT_of, w_dram, grp, bias_sb, agin, row_off):
            """Streamed o-sharded matmul: out.T[o8, B] = W_shard @ act (+bias),
            written o-major (bf16) into agin[row_off : row_off+o8, :]."""
            nslice = (o8 + 391) // 392
            pacc = acc_pool.tile([B, 512 * nslice], F32, tag="acc", name="pacc")
            for g in range(0, nk, grp):
                pool = wtd_pool if o8 == OTD8 else wbu_pool
                wt = pool.tile([KP, grp, o8], F32R, tag="w", name="wt")
                nc.sync.dma_start(wt[:], w_dram[g:g + grp].rearrange("k p o -> p k o"))
                for j in range(grp):
                    k = g + j
                    for sl in range(nslice):
                        o0 = sl * 392
                        ln = min(392, o8 - o0)
                        nc.tensor.matmul(
                            pacc[:, sl * 512: sl * 512 + ln],
                            lhsT_of(k).opt(),
                            wt[:, j, o0:o0 + ln].opt(),
                            start=(k == 0), stop=(k == nk - 1),
                        )
            sba = sbacc_pool.tile([B, o8], F32, tag="sba", name="sba")
            if nslice > 1:
                pv = pacc[:].rearrange("b (s o) -> b s o", s=nslice)[:, :, 0:392]
                sv = sba[:].rearrange("b (s o) -> b s o", s=nslice)
                nc.scalar.activation(sv, pv, AF.Copy)
            else:
                nc.scalar.activation(sba[:], pacc[:, 0:o8], AF.Copy)
            outT = outt_pool.tile([128, nj, B], GDT, tag="outT", name="outT")
            for jj in range(nj):
                w_ = min(128, o8 - jj * 128)
                pt = pst_pool.tile([128, HID], F32, tag="psT", name="pt2")
                nc.tensor.transpose(pt[:w_, 0:B], sba[:, jj * 128: jj * 128 + w_],
                                    ident[0:B, 0:B])
                nc.scalar.activation(outT[:w_, jj, :], pt[:w_, 0:B], AF.Identity,
                                     bias=bias_sb[0:w_, jj:jj + 1])
                nc.gpsimd.dma_start(
                    agin[row_off + jj * 128: row_off + jj * 128 + w_, :],
                    outT[:w_, jj, :])

        def do_gather(agin, agout, nrows):
            if no_cc:
                for c in range(NCORES):
                    nc.gpsimd.dma_start(agout[c], agin[:])
            else:
                nc.gpsimd.collective_compute(
                    "AllGather", mybir.AluOpType.bypass,
                    replica_groups=[list(range(NCORES))],
                    ins=[agin.opt()], outs=[agout.opt()])

        def reload(buf, agout, row_off, nch_l):
            """agout [8, rows, B] o-major (bf16) -> buf [8*nch_l, 16, 16, B]."""
            for c in range(NCORES):
                src = agout[c, row_off: row_off + nch_l * 196, :].rearrange(
                    "(chl y x) b -> chl y x b", chl=nch_l, y=14, x=14)
                nc.gpsimd.dma_start(
                    buf[nch_l * c: nch_l * (c + 1), 1:15, 1:15, :], src)

        def conv(inp, w_ap_of, nco, bias_ap, out_t, act_fn):
            inp_f = inp[:].rearrange("c y x b -> c (y x b)")
            out_f = out_t.rearrange("c y x b -> c (y x b)")
            q = 0
            while q < WLEN:
                ln = min(512, WLEN - q)
                pc = conv_pool.tile([nco, 512], F32, tag="conv", name="pc")
                for i in range(9):
                    dy, dx = SHIFTS[i]
                    off = (dy * XP + dx) * B
                    nc.tensor.matmul(
                        pc[:, 0:ln],
                        w_ap_of(i).opt(),
                        inp_f[:, WSTART + q + off: WSTART + q + off + ln],
                        start=(i == 0), stop=(i == 8),
                    )
                nc.scalar.activation(out_f[:, WSTART + q: WSTART + q + ln],
                                     pc[:, 0:ln], act_fn, bias=bias_ap)
                q += ln

        def cell(node, td_t):
            """GRU cell update of h[node] from bu_buf (+ td_t)."""
            hh = h[node]
            nc.vector.tensor_copy(comb[0:HID, :, :, :], hh[:])
            nc.vector.tensor_copy(comb[HID:CIN, :, :, :], bu_buf[:])
            if td_t is not None:
                nc.vector.tensor_add(comb[:], comb[:], td_t[:])
            conv(comb, lambda i: wg_sb[:, node, i, :], 2 * HID,
                 bg_sb[:, node:node + 1], rz[:], AF.Sigmoid)
            # comb -> cand-conv input: [r*h, bu]
            nc.vector.tensor_mul(comb[0:HID, :, :, :], rz[0:HID, :, :, :], hh[:])
            if td_t is not None:
                nc.vector.tensor_copy(comb[HID:CIN, :, :, :], bu_buf[:])
            # cand -> rz[0:HID] (r no longer needed)
            conv(comb, lambda i: wc_sb[:, node, i, :], HID,
                 bc_sb[:, node:node + 1], rz[0:HID, :, :, :], AF.Tanh)
            hv = hh[:, 1:15, 1:15, :]
            cv = rz[0:HID, 1:15, 1:15, :]
            # z lives at base partition 32; DVE tensor-tensor ops need equal
            # base partitions, so stage it at base 0 in comb (free now).
            zc = comb[0:HID, 1:15, 1:15, :]
            nc.vector.tensor_copy(zc, rz[HID:2 * HID, 1:15, 1:15, :])
            nc.vector.tensor_sub(cv, cv, hv)
            nc.vector.tensor_mul(cv, cv, zc)
            nc.vector.tensor_add(hv, hv, cv)

        # ------------- round schedule: 2 collectives per timestep -------------
        # Round u (u = timestep of cell1/cell2):
        #   B(u): gather {td1(u) [u>=2], bu1(u)} -> cell1(u)
        #   A(u): gather {bu2(u) [u>=1], bu0(u+1) [u+1<T], td0(u+1) [2<=u+1<T]}
        #         -> cell2(u) [u>=1], cell0(u+1) [u+1<T]
        # bu2(u) and td0(u+1) share mp(h1@u) (one maxpool+transpose).
        def round_A(u):
            hbu2 = 1 <= u < t_end
            hbu0 = u + 1 < min(T, t_end)
            htd0 = 2 <= u + 1 < min(T, t_end)
            rows = (OBU8 if hbu2 else 0) + (OBU8 if hbu0 else 0) + (OTD8 if htd0 else 0)
            if rows == 0:
                return
            agin = dram.tile([rows, B], GDT, name=f"aginA_{u}", tag=f"aginA_{u}")
            ro = 0
            m1 = maxpool_transpose(h[1]) if (hbu2 or htd0) else None
            ro_bu2 = ro
            if hbu2:
                big_matmul(KH, OBU8, NJ_BU,
                           lambda k, m=m1: m[k % 2][:, (k // 2), :], bw2, GRP_BU,
                           bb2_sb, agin, ro)
                ro += OBU8
            ro_bu0 = ro
            if hbu0:
                xt_t = mpt_pool.tile([KP, KX, B], F32R, tag="xt", name="xt_t", bufs=2)
                nc.sync.dma_start(xt_t[:], xt_in[u + 1].rearrange("k p b -> p k b"))
                big_matmul(KX, OBU8, NJ_BU,
                           lambda k: xt_t[:, k, :], bw0, 3,
                           bb0_sb, agin, ro)
                ro += OBU8
            ro_td0 = ro
            if htd0:
                big_matmul(KH, OTD8, NJ_TD,
                           lambda k, m=m1: m[k % 2][:, (k // 2), :], tw0, GRP_TD,
                           tb0_sb, agin, ro)
                ro += OTD8
            agout = dram.tile([NCORES, rows, B], GDT, name=f"agoutA_{u}",
                              tag=f"agoutA_{u}",
                              addr_space="Local" if no_cc else "Shared")
            do_gather(agin, agout, 0)
            if htd0:
                reload(td_buf[0], agout, ro_td0, CIN // NCORES)
            if hbu2:
                reload(bu_buf, agout, ro_bu2, IND // NCORES)
                cell(2, None)
            if hbu0:
                reload(bu_buf, agout, ro_bu0, IND // NCORES)
                cell(0, td_buf[0] if htd0 else None)

        def round_B(u):
            htd1 = u >= 2
            rows = (OTD8 if htd1 else 0) + OBU8
            agin = dram.tile([rows, B], GDT, name=f"aginB_{u}", tag=f"aginB_{u}")
            ro = 0
            if htd1:
                m2 = maxpool_transpose(h[2])
                big_matmul(KH, OTD8, NJ_TD,
                           lambda k, m=m2: m[k % 2][:, (k // 2), :], tw1, GRP_TD,
                           tb1_sb, agin, ro)
                ro += OTD8
            m0 = maxpool_transpose(h[0])
            big_matmul(KH, OBU8, NJ_BU,
                       lambda k, m=m0: m[k % 2][:, (k // 2), :], bw1, GRP_BU,
                       bb1_sb, agin, ro)
            agout = dram.tile([NCORES, rows, B], GDT, name=f"agoutB_{u}",
                              tag=f"agoutB_{u}",
                              addr_space="Local" if no_cc else "Shared")
            do_gather(agin, agout, 0)
            if htd1:
                reload(td_buf[1], agout, 0, CIN // NCORES)
            reload(bu_buf, agout, ro, IND // NCORES)
            cell(1, td_buf[1] if htd1 else None)

        round_A(-1)                      # bootstrap: bu0(0) -> cell0(0)
        for u in range(t_end):
            if u >= 1:
                round_B(u)
            round_A(u)

        if debug_h:
            for i in range(N):
                nc.gpsimd.dma_start(dbg_ext[i], h[i][:, 1:15, 1:15, :])
        # ---------------- final FC head ----------------
        nc.scalar.activation(rz[0:HID, :, :, :], h[2][:], AF.Relu)
        pT = transpose_feat(rz)
        pfc = acc_pool.tile([100, 16], F32, tag="acc", name="pfc")
        for g in range(0, KH, 8):
            wf = wtd_pool.tile([KP, 8, 100], F32R, tag="w", name="wf")
            nc.sync.dma_start(wf[:], fc1_in[g:g + 8].rearrange("k p o -> p k o"))
            for j in range(8):
                k = g + j
                nc.tensor.matmul(pfc[:], wf[:, j, :].opt(), pT[k % 2][:, (k // 2), :].opt(),
                                 start=(k == 0), stop=(k == KH - 1))
        p1 = sbacc_pool.tile([100, 16], F32, tag="sba", name="p1")
        nc.scalar.activation(p1[:], pfc[:], AF.Relu, bias=fc1b_sb[:])
        pf2 = pst_pool.tile([128, HID], F32, tag="psT", name="pf2")
        nc.tensor.matmul(pf2[0:10, 0:16], fc2_sb[:], p1[:], start=True, stop=True)
        osb = sbacc_pool.tile([10, 16], F32, tag="osb", name="osb")
        nc.scalar.activation(osb[:], pf2[0:10, 0:16], AF.Identity, bias=fc2b_sb[:])
        nc.gpsimd.dma_start(out_ext[:], osb[:])

    nc.finalize()
    return nc


# ---------------------------------------------------------------- host ----
def _feat_perm(nch):
    """Device feature order (ch, s, p) -> torch flat feature index."""
    perm = np.zeros((nch * 2, KP), np.int64)
    for ch in range(nch):
        for s in range(2):
            k = ch * 2 + s
            p = np.arange(KP)
            y = s * 7 + p // 14
            x = p % 14
            perm[k] = ch * 196 + y * 14 + x
    return perm


def _shard_w(wmat, nch_in, o8):
    """wmat (O, K) torch-order -> per-core [nk, 98, o8] streaming shards."""
    perm = _feat_perm(nch_in)
    wt = wmat.T[perm.reshape(-1)].reshape(perm.shape[0], KP, wmat.shape[0])
    return [np.ascontiguousarray(wt[:, :, c * o8:(c + 1) * o8]) for c in range(NCORES)]


def _pad_bias(bvec, o8, nj):
    out = []
    for c in range(NCORES):
        bp = np.zeros((nj, 128), np.float32)
        bp.reshape(-1)[:o8] = bvec[c * o8:(c + 1) * o8]
        out.append(bp)
    return out


def prep_inputs(inputs):
    x = np.asarray(inputs["x"], np.float32)
    permx = _feat_perm(C)
    xt = np.zeros((T, KX, KP, B), np.float32)
    for t in range(T):
        flat = x[:, t].reshape(B, C * 196).T      # [588, B]
        xt[t] = flat[permx.reshape(-1)].reshape(KX, KP, B)

    # td outputs are reloaded straight into device channel order [h, bu]:
    # permute td_w / td_b output rows from torch order [bu, h] accordingly.
    ci_out = np.concatenate([np.arange(IND, CIN), np.arange(0, IND)])
    o_perm = (ci_out[:, None] * 196 + np.arange(196)[None, :]).reshape(-1)
    tw0 = _shard_w(np.asarray(inputs["td_w0"], np.float32)[o_perm], HID, OTD8)
    tw1 = _shard_w(np.asarray(inputs["td_w1"], np.float32)[o_perm], HID, OTD8)
    bw0 = _shard_w(np.asarray(inputs["bu_w0"], np.float32), C, OBU8)
    bw1 = _shard_w(np.asarray(inputs["bu_w1"], np.float32), HID, OBU8)
    bw2 = _shard_w(np.asarray(inputs["bu_w2"], np.float32), HID, OBU8)
    tb0 = _pad_bias(np.asarray(inputs["td_b0"], np.float32)[o_perm], OTD8, NJ_TD)
    tb1 = _pad_bias(np.asarray(inputs["td_b1"], np.float32)[o_perm], OTD8, NJ_TD)
    bb0 = _pad_bias(np.asarray(inputs["bu_b0"], np.float32), OBU8, NJ_BU)
    bb1 = _pad_bias(np.asarray(inputs["bu_b1"], np.float32), OBU8, NJ_BU)
    bb2 = _pad_bias(np.asarray(inputs["bu_b2"], np.float32), OBU8, NJ_BU)

    # conv weights: device ci order = [h (0:32) -> torch ci 16..47, bu -> 0..15]
    ci_perm = np.concatenate([np.arange(IND, CIN), np.arange(0, IND)])
    Wg = np.asarray(inputs["Wg"], np.float32)
    Wc = np.asarray(inputs["Wc"], np.float32)
    wg = np.zeros((N, 9, CIN, 2 * HID), np.float32)
    wc = np.zeros((N, 9, CIN, HID), np.float32)
    for si, (dy, dx) in enumerate(SHIFTS):
        for n in range(N):
            wg[n, si] = Wg[n][:, ci_perm, dy + 1, dx + 1].T
            wc[n, si] = Wc[n][:, ci_perm, dy + 1, dx + 1].T

    permh = _feat_perm(HID)
    fc1 = np.asarray(inputs["fc1_w"], np.float32)     # (100, 6272)
    fc1t = np.ascontiguousarray(fc1.T[permh.reshape(-1)].reshape(KH, KP, 100))
    fc2t = np.ascontiguousarray(np.asarray(inputs["fc2_w"], np.float32).T)  # (100, 10)

    common = {
        "xt": xt,
        "wg": wg, "wc": wc,
        "bg": np.asarray(inputs["bg"], np.float32),
        "bc": np.asarray(inputs["bc"], np.float32),
        "fc1t": fc1t,
        "fc1b": np.asarray(inputs["fc1_b"], np.float32).reshape(100, 1),
        "fc2t": fc2t,
        "fc2b": np.asarray(inputs["fc2_b"], np.float32).reshape(10, 1),
        "ident": np.eye(128, dtype=np.float32),
    }
    in_maps = []
    for c in range(NCORES):
        m = dict(common)
        m.update({
            "tw0": tw0[c], "tw1": tw1[c], "bw0": bw0[c],
            "bw1": bw1[c], "bw2": bw2[c],
            "tb0": tb0[c], "tb1": tb1[c],
            "bb0": bb0[c], "bb1": bb1[c], "bb2": bb2[c],
        })
        in_maps.append(m)
    return in_maps


def get_graph():
    if "nc" not in _CACHED:
        _CACHED["nc"] = build_graph()
    return _CACHED["nc"]


def kernel(**inputs):
    nc = get_graph()
    in_maps = prep_inputs(inputs)
    res = bass_utils.run_bass_kernel_spmd(nc, in_maps, core_ids=list(range(NCORES)))
    out_t = np.asarray(res.results[0]["out"]).reshape(10, B)
    return np.ascontiguousarray(out_t.T).astype(np.float32)

